# revision 6
# baseline (speedup 1.0000x reference)
"""Trainium2 Bass kernel for nn_Decision_Node (Linear+Hardtanh -> sp, 2-class
softmax Gini -> gini), data-parallel over 8 NeuronCores.

Math per core shard (B_s=128 of B=1024 batches, T=128, F=784, L=256, C=2):
    sp   = clip(x @ W.T + b, -1, 1)                      [N=16384, 256]
    p0   = sigmoid(sp * d),  d = contrib[...,0]-contrib[...,1]
    gini = 2 - p0^2 - p1^2 = 1.5 - 0.5*tanh(sp*d/2)^2

Device strategy (flipped layout: L on partitions, rows on free dim):
  - Stationary operand = W chunks [K=128, M=128]; moving operand =
    transposed-x tiles [K, N=512]. Bias folded as the 17th row of the
    last (K=17) contraction tile. 8 PSUM banks cycle the k=0..6
    accumulation so the PE never idles (no HAM re-throttle).
  - x is staged in DRAM chunk-major ([128, 6, ch] contiguous per chunk)
    so each chunk is ONE big DMA per queue half (24 KiB/partition lines),
    split across the sync + gpsimd queues; outputs ride the scalar queue.
  - DVE: fused hardtanh clip (PSUM drain), z = sp*d, sp uint8 quantize.
  - ACT: tanh(z/2); Square(sqrt(255)*th) -> u8 gini in one op.
  - Outputs u8, combined in one [128, (kind,lc,ch)] staging tile ->
    single DMA per chunk; host de-quantizes/transposes.
"""

import os
import sys
import types
from concurrent.futures import ThreadPoolExecutor

import numpy as np

for _p in (
    "/opt/trn_rl_repo",
    "/root/.axon_site",
    "/root/.axon_site/_ro/trn_rl_repo",
    "/root/.axon_site/_ro/pypackages",
):
    if os.path.isdir(_p) and _p not in sys.path:
        sys.path.append(_p)

B, T, F, L = 1024, 128, 784, 256
NCORES = 8
BS = B // NCORES          # batches per core
NROWS = BS * T            # 16384 rows per core
KT = 7                    # contraction tiles (784 = 6*128 + 16, + bias row)
KP = 17                   # contraction rows in the last k-tile (16 + bias)
CH = 2048                 # max rows per pipeline chunk
BANK = 512                # rows per PSUM bank / matmul free size
CHUNKS = (512, 1536) + (2048,) * 6 + (1536, 512)
PSPLIT = 64               # x-tile partitions on the sync queue (rest gpsimd)


def _build_module():
    import concourse.tile as tile
    from concourse import bacc, mybir

    f32, f16, u8 = mybir.dt.float32, mybir.dt.float16, mybir.dt.uint8
    Alu = mybir.AluOpType
    Act = mybir.ActivationFunctionType

    nc = bacc.Bacc(
        "TRN2",
        target_bir_lowering=False,
        debug=False,
        enable_asserts=False,
        num_devices=NCORES,
    )
    # x k-tiles 0..5, chunk-major contiguous: [p, 6*n0 + k*ch + j]
    xm_d = nc.dram_tensor("xm", [128, 6 * NROWS], f16, kind="ExternalInput").ap()
    # x k-tile 6 (16 remainder features + all-ones bias row)
    x6_d = nc.dram_tensor("x6", [KP, NROWS], f16, kind="ExternalInput").ap()
    wt_d = nc.dram_tensor("wt", [KT, 128, L], f16, kind="ExternalInput").ap()
    dr_d = nc.dram_tensor("dr", [2, 128, T], f16, kind="ExternalInput").ap()
    # combined u8 outputs: [kind (0=sp,1=gini), lc, l, n]
    oq_d = nc.dram_tensor("oq", [2, 2, 128, NROWS], u8, kind="ExternalOutput").ap()

    SQ255 = float(np.float32(np.sqrt(255.0)))

    with tile.TileContext(nc) as tc:
        with (
            tc.tile_pool(name="consts", bufs=1) as consts,
            tc.tile_pool(name="xt", bufs=3) as xt_pool,
            tc.tile_pool(name="psum", bufs=8, space="PSUM") as psum_pool,
            tc.tile_pool(name="sp", bufs=2) as sp_pool,
            tc.tile_pool(name="tmp", bufs=2) as tmp_pool,
            tc.tile_pool(name="outq", bufs=3) as outq_pool,
        ):
            wt_sb = consts.tile([128, KT, L], f16)
            nc.scalar.dma_start(wt_sb[:], wt_d.rearrange("k p l -> p k l"))
            dr_sb = consts.tile([128, 2, CH], f16)
            nc.scalar.dma_start(dr_sb[:, :, 0:T], dr_d.rearrange("c p n -> p c n"))
            # replicate d along the row axis: [*, lc, 0:128] -> [*, lc, 0:2048]
            w = T
            while w < CH:
                for lc in range(2):
                    nc.vector.tensor_scalar(
                        dr_sb[:, lc, w : 2 * w], dr_sb[:, lc, 0:w], 0.0, None, Alu.add
                    )
                w *= 2

            # PE warmup during the initial DMA wait so the HAM clock gate
            # flips to 8/8 right as real work arrives.
            wrm = consts.tile([128, 256], f16, tag="wrm")
            nc.vector.memset(wrm[:], 0.0)
            b128 = consts.tile([128, 1], f32, tag="b128")
            nc.vector.memset(b128[:], 128.0)
            wps = psum_pool.tile([128, BANK], f32, tag="ps", bufs=8)
            for _ in range(12):
                nc.tensor.matmul(
                    wps[:, 0:256], wrm[:, 0:128], wrm[:], start=True, stop=True
                )
            wsink = consts.tile([128, 1], f16, tag="wsink")
            nc.vector.tensor_scalar(wsink[:], wps[:, 0:1], 0.0, None, Alu.mult)

            n0 = 0
            for ci, ch in enumerate(CHUNKS):
                nb = ch // BANK
                xm = xt_pool.tile([128, 6, CH], f16, tag="xm", bufs=3)
                src = xm_d[:, 6 * n0 : 6 * n0 + 6 * ch].rearrange(
                    "p (k j) -> p k j", k=6
                )
                nc.sync.dma_start(xm[0:PSPLIT, :, 0:ch], src[0:PSPLIT])
                nc.gpsimd.dma_start(xm[PSPLIT:128, :, 0:ch], src[PSPLIT:128])
                x6 = xt_pool.tile([KP, CH], f16, tag="x6", bufs=3)
                nc.gpsimd.dma_start(x6[:, 0:ch], x6_d[:, n0 : n0 + ch])

                sp16 = sp_pool.tile([128, 2, CH], f16, tag="sp", bufs=2)
                for lc in range(2):
                    for bi in range(nb):
                        ps = psum_pool.tile([128, BANK], f32, tag="ps", bufs=8)
                        for k in range(6):
                            nc.tensor.matmul(
                                ps[:],
                                wt_sb[:, k, lc * 128 : (lc + 1) * 128],
                                xm[:, k, bi * BANK : (bi + 1) * BANK],
                                start=(k == 0),
                                stop=False,
                            )
                        nc.tensor.matmul(
                            ps[:],
                            wt_sb[0:KP, 6, lc * 128 : (lc + 1) * 128],
                            x6[:, bi * BANK : (bi + 1) * BANK],
                            start=False,
                            stop=True,
                        )
                        # fused hardtanh: (ps max -1) min 1, PSUM -> SBUF f16
                        nc.vector.tensor_scalar(
                            sp16[:, lc, bi * BANK : (bi + 1) * BANK],
                            ps[:],
                            -1.0,
                            1.0,
                            Alu.max,
                            Alu.min,
                        )
                z = tmp_pool.tile([128, 2, CH], f16, tag="z", bufs=2)
                nc.vector.tensor_tensor(
                    z[:, :, 0:ch], sp16[:, :, 0:ch], dr_sb[:, :, 0:ch], Alu.mult
                )
                th = tmp_pool.tile([128, 2, CH], f16, tag="th", bufs=2)
                nc.scalar.activation(th[:, :, 0:ch], z[:, :, 0:ch], Act.Tanh, scale=0.5)
                oq = outq_pool.tile([128, 2, 2, CH], u8, tag="oq", bufs=3)
                nc.scalar.activation(
                    oq[:, 1, :, 0:ch], th[:, :, 0:ch], Act.Square, scale=SQ255
                )
                nc.vector.tensor_scalar(
                    oq[:, 0, :, 0:ch], sp16[:, :, 0:ch], 127.5, 128.0, Alu.mult, Alu.add
                )
                nc.scalar.dma_start(
                    oq_d[:, :, :, n0 : n0 + ch].rearrange("a c p n -> p a c n"),
                    oq[:, :, :, 0:ch],
                )
                n0 += ch

    nc.compile()
    return nc


def _prep_core_x(x_flat_core):
    """[16384, 784] fp32 -> (xm [128, 6*16384] chunk-major f16, x6 [17, 16384])."""
    n = x_flat_core.shape[0]
    xsT16 = x_flat_core.T.astype(np.float16)  # [784, n], one strided pass
    main = xsT16[:768].reshape(6, 128, n)
    xm = np.empty((128, 6 * n), np.float16)
    n0 = 0
    for ch in CHUNKS:
        blk = main[:, :, n0 : n0 + ch]  # [6, 128, ch]
        xm[:, 6 * n0 : 6 * n0 + 6 * ch] = blk.transpose(1, 0, 2).reshape(128, 6 * ch)
        n0 += ch
    x6 = np.empty((KP, n), np.float16)
    x6[:16] = xsT16[768:784]
    x6[16] = 1.0
    return xm, x6


def _prep_wt(W, b):
    wt = np.zeros((KT, 128, L), np.float16)
    WT = W.T  # [784, 256]
    for k in range(6):
        wt[k] = WT[k * 128 : (k + 1) * 128]
    wt[6, :16] = WT[768:784]
    wt[6, 16] = b
    return wt


_module_cache = {}


def _get_module():
    if "m" not in _module_cache:
        _module_cache["m"] = _build_module()
    return _module_cache["m"]


def _install_ntff_hook():
    """Register the axon NTFF profiling hook missing from this image's antenv."""
    try:
        import antenv.axon_hooks  # noqa: F401

        return
    except ImportError:
        pass
    try:
        from trn_agent_boot.trn_boot import _ntff_profile_via_ctypes

        hook = _ntff_profile_via_ctypes("/opt/axon/libaxon_pjrt.so")
    except Exception:
        hook = None
    mod = types.ModuleType("antenv.axon_hooks")
    mod.get_axon_ntff_profile_hook = lambda: hook
    mod.set_axon_ntff_profile_hook = lambda h: None
    sys.modules["antenv.axon_hooks"] = mod


def _unstage(oq_raw):
    """[2, 2, 128, 16384] u8 -> (sp, gini) [16384, 256] fp32."""
    spq = np.ascontiguousarray(oq_raw[0].transpose(2, 0, 1).reshape(NROWS, L))
    giq = np.ascontiguousarray(oq_raw[1].transpose(2, 0, 1).reshape(NROWS, L))
    sp = spq.astype(np.float32)
    sp -= 127.5
    sp *= 1.0 / 127.5
    gini = giq.astype(np.float32)
    gini *= -0.5 / 255.0
    gini += 1.5
    return sp, gini


def _run(x, W, b, contribution, trace=False, tmpdir=None):
    from concourse import bass_utils

    nc = _get_module()

    x_flat = np.ascontiguousarray(x, dtype=np.float32).reshape(NCORES, NROWS, F)
    wt = _prep_wt(np.asarray(W, np.float32), np.asarray(b, np.float32))
    c = np.asarray(contribution, np.float32)
    d = np.ascontiguousarray(c[:, :, 0] - c[:, :, 1], dtype=np.float32)
    dr = np.ascontiguousarray(d.T.astype(np.float16).reshape(2, 128, T))

    with ThreadPoolExecutor(NCORES) as ex:
        xs = list(ex.map(_prep_core_x, [x_flat[i] for i in range(NCORES)]))

    if trace:
        _install_ntff_hook()
    in_maps = [
        {"xm": xs[i][0], "x6": xs[i][1], "wt": wt, "dr": dr} for i in range(NCORES)
    ]
    res = bass_utils.run_bass_kernel_spmd(
        nc, in_maps, core_ids=list(range(NCORES)), trace=trace, tmpdir=tmpdir
    )

    with ThreadPoolExecutor(NCORES) as ex:
        outs = list(ex.map(lambda i: _unstage(res.results[i]["oq"]), range(NCORES)))
    sp = np.concatenate([o[0] for o in outs]).reshape(B, T, L)
    gini = np.concatenate([o[1] for o in outs]).reshape(B, T, L)
    out = (sp, gini)
    return (out, res) if trace else (out, None)


def kernel(x, W, b, contribution):
    out, _ = _run(x, W, b, contribution, trace=False)
    return out


# revision 7
# speedup vs baseline: 1.1115x; 1.1115x over previous
"""Trainium2 Bass kernel for nn_Decision_Node (Linear+Hardtanh -> sp, 2-class
softmax Gini -> gini), data-parallel over 8 NeuronCores.

Math per core shard (B_s=128 of B=1024 batches, T=128, F=784, L=256, C=2):
    sp   = clip(x @ W.T + b, -1, 1)                      [N=16384, 256]
    p0   = sigmoid(sp * d),  d = contrib[...,0]-contrib[...,1]
    gini = 2 - p0^2 - p1^2 = 1.5 - 0.5*tanh(sp*d/2)^2

Device strategy (flipped layout: L on partitions, rows on free dim):
  - Stationary operand = W chunks [K=128, M=128]; moving operand =
    transposed-x tiles [K, N=512]. Bias folded as the 17th row of the
    last (K=17) contraction tile. 8 PSUM banks cycle the k=0..6
    accumulation so the PE never idles (no HAM re-throttle).
  - x is staged in DRAM chunk-major ([128, 6, ch] contiguous per chunk)
    so each chunk is ONE big DMA per queue half (24 KiB/partition lines),
    split across the sync + gpsimd queues; outputs ride the scalar queue.
  - DVE: fused hardtanh clip (PSUM drain), z = sp*d, sp uint8 quantize.
  - ACT: tanh(z/2); Square(sqrt(255)*th) -> u8 gini in one op.
  - Outputs u8, combined in one [128, (kind,lc,ch)] staging tile ->
    single DMA per chunk; host de-quantizes/transposes.
"""

import os
import sys
import types
from concurrent.futures import ThreadPoolExecutor

import numpy as np

for _p in (
    "/opt/trn_rl_repo",
    "/root/.axon_site",
    "/root/.axon_site/_ro/trn_rl_repo",
    "/root/.axon_site/_ro/pypackages",
):
    if os.path.isdir(_p) and _p not in sys.path:
        sys.path.append(_p)

B, T, F, L = 1024, 128, 784, 256
NCORES = 8
BS = B // NCORES          # batches per core
NROWS = BS * T            # 16384 rows per core
KT = 7                    # contraction tiles (784 = 6*128 + 16, + bias row)
KP = 17                   # contraction rows in the last k-tile (16 + bias)
CH = 2048                 # max rows per pipeline chunk
BANK = 512                # rows per PSUM bank / matmul free size
CHUNKS = (512, 1536) + (2048,) * 6 + (1536, 512)


def _build_module():
    import concourse.tile as tile
    from concourse import bacc, mybir

    f32, f16, u8 = mybir.dt.float32, mybir.dt.float16, mybir.dt.uint8
    Alu = mybir.AluOpType
    Act = mybir.ActivationFunctionType

    nc = bacc.Bacc(
        "TRN2",
        target_bir_lowering=False,
        debug=False,
        enable_asserts=False,
        num_devices=NCORES,
    )
    xt_d = nc.dram_tensor("xt", [KT, 128, NROWS], f16, kind="ExternalInput").ap()
    wt_d = nc.dram_tensor("wt", [KT, 128, L], f16, kind="ExternalInput").ap()
    dr_d = nc.dram_tensor("dr", [2, 128, T], f16, kind="ExternalInput").ap()
    # combined u8 outputs: [kind (0=sp,1=gini), lc, l, n]
    oq_d = nc.dram_tensor("oq", [2, 2, 128, NROWS], u8, kind="ExternalOutput").ap()

    SQ255 = float(np.float32(np.sqrt(255.0)))

    with tile.TileContext(nc) as tc:
        with (
            tc.tile_pool(name="consts", bufs=1) as consts,
            tc.tile_pool(name="xt", bufs=3) as xt_pool,
            tc.tile_pool(name="psum", bufs=8, space="PSUM") as psum_pool,
            tc.tile_pool(name="sp", bufs=2) as sp_pool,
            tc.tile_pool(name="tmp", bufs=2) as tmp_pool,
            tc.tile_pool(name="outq", bufs=3) as outq_pool,
        ):
            wt_sb = consts.tile([128, KT, L], f16)
            nc.scalar.dma_start(wt_sb[:], wt_d.rearrange("k p l -> p k l"))
            dr_sb = consts.tile([128, 2, CH], f16)
            nc.scalar.dma_start(dr_sb[:, :, 0:T], dr_d.rearrange("c p n -> p c n"))
            # replicate d along the row axis: [*, lc, 0:128] -> [*, lc, 0:2048]
            w = T
            while w < CH:
                for lc in range(2):
                    nc.vector.tensor_scalar(
                        dr_sb[:, lc, w : 2 * w], dr_sb[:, lc, 0:w], 0.0, None, Alu.add
                    )
                w *= 2

            # PE warmup during the initial DMA wait so the HAM clock gate
            # flips to 8/8 right as real work arrives.
            wrm = consts.tile([128, 256], f16, tag="wrm")
            nc.vector.memset(wrm[:], 0.0)
            b128 = consts.tile([128, 1], f32, tag="b128")
            nc.vector.memset(b128[:], 128.0)
            wps = psum_pool.tile([128, BANK], f32, tag="ps", bufs=8)
            for _ in range(12):
                nc.tensor.matmul(
                    wps[:, 0:256], wrm[:, 0:128], wrm[:], start=True, stop=True
                )
            wsink = consts.tile([128, 1], f16, tag="wsink")
            nc.vector.tensor_scalar(wsink[:], wps[:, 0:1], 0.0, None, Alu.mult)

            n0 = 0
            for ci, ch in enumerate(CHUNKS):
                nb = ch // BANK
                xks = []
                for k in range(6):
                    xk = xt_pool.tile([128, CH], f16, tag=f"x{k}", bufs=3)
                    eng = nc.sync if k % 2 == 0 else nc.gpsimd
                    eng.dma_start(xk[:, 0:ch], xt_d[k, :, n0 : n0 + ch])
                    xks.append(xk)
                x6 = xt_pool.tile([KP, CH], f16, tag="x6", bufs=3)
                nc.gpsimd.dma_start(x6[:, 0:ch], xt_d[6, 0:KP, n0 : n0 + ch])

                sp16 = sp_pool.tile([128, 2, CH], f16, tag="sp", bufs=2)
                for lc in range(2):
                    for bi in range(nb):
                        ps = psum_pool.tile([128, BANK], f32, tag="ps", bufs=8)
                        for k in range(6):
                            nc.tensor.matmul(
                                ps[:],
                                wt_sb[:, k, lc * 128 : (lc + 1) * 128],
                                xks[k][:, bi * BANK : (bi + 1) * BANK],
                                start=(k == 0),
                                stop=False,
                            )
                        nc.tensor.matmul(
                            ps[:],
                            wt_sb[0:KP, 6, lc * 128 : (lc + 1) * 128],
                            x6[:, bi * BANK : (bi + 1) * BANK],
                            start=False,
                            stop=True,
                        )
                        # fused hardtanh: (ps max -1) min 1, PSUM -> SBUF f16
                        nc.vector.tensor_scalar(
                            sp16[:, lc, bi * BANK : (bi + 1) * BANK],
                            ps[:],
                            -1.0,
                            1.0,
                            Alu.max,
                            Alu.min,
                        )
                z = tmp_pool.tile([128, 2, CH], f16, tag="z", bufs=2)
                nc.vector.tensor_tensor(
                    z[:, :, 0:ch], sp16[:, :, 0:ch], dr_sb[:, :, 0:ch], Alu.mult
                )
                th = tmp_pool.tile([128, 2, CH], f16, tag="th", bufs=2)
                nc.scalar.activation(th[:, :, 0:ch], z[:, :, 0:ch], Act.Tanh, scale=0.5)
                oq = outq_pool.tile([128, 2, 2, CH], u8, tag="oq", bufs=3)
                nc.scalar.activation(
                    oq[:, 1, :, 0:ch], th[:, :, 0:ch], Act.Square, scale=SQ255
                )
                nc.vector.tensor_scalar(
                    oq[:, 0, :, 0:ch], sp16[:, :, 0:ch], 127.5, 128.0, Alu.mult, Alu.add
                )
                nc.scalar.dma_start(
                    oq_d[:, :, :, n0 : n0 + ch].rearrange("a c p n -> p a c n"),
                    oq[:, :, :, 0:ch],
                )
                n0 += ch

    nc.compile()
    return nc


def _prep_core_x(x_flat_core):
    """[16384, 784] fp32 -> transposed fp16 [7, 128, 16384] (f on partitions).

    Row 16 of the last k-tile is the all-ones bias-fold row.
    """
    n = x_flat_core.shape[0]
    xsT16 = x_flat_core.T.astype(np.float16)  # [784, n], one strided pass
    xt = np.zeros((KT, 128, n), np.float16)
    xt[:6] = xsT16[:768].reshape(6, 128, n)
    xt[6, :16] = xsT16[768:784]
    xt[6, 16] = 1.0
    return xt


def _prep_wt(W, b):
    wt = np.zeros((KT, 128, L), np.float16)
    WT = W.T  # [784, 256]
    for k in range(6):
        wt[k] = WT[k * 128 : (k + 1) * 128]
    wt[6, :16] = WT[768:784]
    wt[6, 16] = b
    return wt


_module_cache = {}


def _get_module():
    if "m" not in _module_cache:
        _module_cache["m"] = _build_module()
    return _module_cache["m"]


def _install_ntff_hook():
    """Register the axon NTFF profiling hook missing from this image's antenv."""
    try:
        import antenv.axon_hooks  # noqa: F401

        return
    except ImportError:
        pass
    try:
        from trn_agent_boot.trn_boot import _ntff_profile_via_ctypes

        hook = _ntff_profile_via_ctypes("/opt/axon/libaxon_pjrt.so")
    except Exception:
        hook = None
    mod = types.ModuleType("antenv.axon_hooks")
    mod.get_axon_ntff_profile_hook = lambda: hook
    mod.set_axon_ntff_profile_hook = lambda h: None
    sys.modules["antenv.axon_hooks"] = mod


def _unstage(oq_raw):
    """[2, 2, 128, 16384] u8 -> (sp, gini) [16384, 256] fp32."""
    spq = np.ascontiguousarray(oq_raw[0].transpose(2, 0, 1).reshape(NROWS, L))
    giq = np.ascontiguousarray(oq_raw[1].transpose(2, 0, 1).reshape(NROWS, L))
    sp = spq.astype(np.float32)
    sp -= 127.5
    sp *= 1.0 / 127.5
    gini = giq.astype(np.float32)
    gini *= -0.5 / 255.0
    gini += 1.5
    return sp, gini


def _run(x, W, b, contribution, trace=False, tmpdir=None):
    from concourse import bass_utils

    nc = _get_module()

    x_flat = np.ascontiguousarray(x, dtype=np.float32).reshape(NCORES, NROWS, F)
    wt = _prep_wt(np.asarray(W, np.float32), np.asarray(b, np.float32))
    c = np.asarray(contribution, np.float32)
    d = np.ascontiguousarray(c[:, :, 0] - c[:, :, 1], dtype=np.float32)
    dr = np.ascontiguousarray(d.T.astype(np.float16).reshape(2, 128, T))

    with ThreadPoolExecutor(NCORES) as ex:
        xs = list(ex.map(_prep_core_x, [x_flat[i] for i in range(NCORES)]))

    if trace:
        _install_ntff_hook()
    in_maps = [{"xt": xs[i], "wt": wt, "dr": dr} for i in range(NCORES)]
    res = bass_utils.run_bass_kernel_spmd(
        nc, in_maps, core_ids=list(range(NCORES)), trace=trace, tmpdir=tmpdir
    )

    with ThreadPoolExecutor(NCORES) as ex:
        outs = list(ex.map(lambda i: _unstage(res.results[i]["oq"]), range(NCORES)))
    sp = np.concatenate([o[0] for o in outs]).reshape(B, T, L)
    gini = np.concatenate([o[1] for o in outs]).reshape(B, T, L)
    out = (sp, gini)
    return (out, res) if trace else (out, None)


def kernel(x, W, b, contribution):
    out, _ = _run(x, W, b, contribution, trace=False)
    return out


# revision 9
# speedup vs baseline: 1.1207x; 1.0082x over previous
"""Trainium2 Bass kernel for nn_Decision_Node (Linear+Hardtanh -> sp, 2-class
softmax Gini -> gini), data-parallel over 8 NeuronCores.

Math per core shard (B_s=128 of B=1024 batches, T=128, F=784, L=256, C=2):
    sp   = clip(x @ W.T + b, -1, 1)                      [N=16384, 256]
    p0   = sigmoid(sp * d),  d = contrib[...,0]-contrib[...,1]
    gini = 2 - p0^2 - p1^2 = 1.5 - 0.5*tanh(sp*d/2)^2

Device strategy (flipped layout: L on partitions, rows on free dim):
  - Stationary operand = W chunks [K=128, M=128]; moving operand =
    transposed-x tiles [K, N=512]. Bias folded as the 17th row of the
    last (K=17) contraction tile. 8 PSUM banks cycle the k=0..6
    accumulation so the PE never idles (no HAM re-throttle).
  - x is staged in DRAM chunk-major ([128, 6, ch] contiguous per chunk)
    so each chunk is ONE big DMA per queue half (24 KiB/partition lines),
    split across the sync + gpsimd queues; outputs ride the scalar queue.
  - DVE: fused hardtanh clip (PSUM drain), z = sp*d, sp uint8 quantize.
  - ACT: tanh(z/2); Square(sqrt(255)*th) -> u8 gini in one op.
  - Outputs u8, combined in one [128, (kind,lc,ch)] staging tile ->
    single DMA per chunk; host de-quantizes/transposes.
"""

import os
import sys
import types
from concurrent.futures import ThreadPoolExecutor

import numpy as np

for _p in (
    "/opt/trn_rl_repo",
    "/root/.axon_site",
    "/root/.axon_site/_ro/trn_rl_repo",
    "/root/.axon_site/_ro/pypackages",
):
    if os.path.isdir(_p) and _p not in sys.path:
        sys.path.append(_p)

B, T, F, L = 1024, 128, 784, 256
NCORES = 8
BS = B // NCORES          # batches per core
NROWS = BS * T            # 16384 rows per core
KT = 7                    # contraction tiles (784 = 6*128 + 16, + bias row)
KP = 17                   # contraction rows in the last k-tile (16 + bias)
CH = 2048                 # max rows per pipeline chunk
BANK = 512                # rows per PSUM bank / matmul free size
CHUNKS = (512, 1536) + (2048,) * 6 + (1536, 512)


def _build_module():
    import concourse.tile as tile
    from concourse import bacc, mybir

    f32, f16, u8 = mybir.dt.float32, mybir.dt.float16, mybir.dt.uint8
    Alu = mybir.AluOpType
    Act = mybir.ActivationFunctionType

    nc = bacc.Bacc(
        "TRN2",
        target_bir_lowering=False,
        debug=False,
        enable_asserts=False,
        num_devices=NCORES,
    )
    xt_d = nc.dram_tensor("xt", [KT, 128, NROWS], f16, kind="ExternalInput").ap()
    wt_d = nc.dram_tensor("wt", [KT, 128, L], f16, kind="ExternalInput").ap()
    dr_d = nc.dram_tensor("dr", [2, 128, T], f16, kind="ExternalInput").ap()
    # combined u8 outputs: [kind (0=sp,1=gini), lc, l, n]
    oq_d = nc.dram_tensor("oq", [2, 2, 128, NROWS], u8, kind="ExternalOutput").ap()

    SQ255 = float(np.float32(np.sqrt(255.0)))

    with tile.TileContext(nc) as tc:
        with (
            tc.tile_pool(name="consts", bufs=1) as consts,
            tc.tile_pool(name="xt", bufs=3) as xt_pool,
            tc.tile_pool(name="psum", bufs=8, space="PSUM") as psum_pool,
            tc.tile_pool(name="sp", bufs=2) as sp_pool,
            tc.tile_pool(name="tmp", bufs=2) as tmp_pool,
            tc.tile_pool(name="outq", bufs=3) as outq_pool,
        ):
            wt_sb = consts.tile([128, KT, L], f16)
            nc.scalar.dma_start(wt_sb[:], wt_d.rearrange("k p l -> p k l"))
            dr_sb = consts.tile([128, 2, CH], f16)
            nc.scalar.dma_start(dr_sb[:, :, 0:T], dr_d.rearrange("c p n -> p c n"))
            # replicate d along the row axis: [*, lc, 0:128] -> [*, lc, 0:2048]
            w = T
            while w < CH:
                for lc in range(2):
                    nc.vector.tensor_scalar(
                        dr_sb[:, lc, w : 2 * w], dr_sb[:, lc, 0:w], 0.0, None, Alu.add
                    )
                w *= 2

            # PE warmup during the initial DMA wait so the HAM clock gate
            # flips to 8/8 right as real work arrives.
            wrm = consts.tile([128, BANK], f16, tag="wrm")
            nc.vector.memset(wrm[:], 0.0)
            b128 = consts.tile([128, 1], f32, tag="b128")
            nc.vector.memset(b128[:], 128.0)
            wps = psum_pool.tile([128, BANK], f32, tag="ps", bufs=8)
            for _ in range(12):
                nc.tensor.matmul(
                    wps[:], wrm[:, 0:128], wrm[:], start=True, stop=True
                )
            wsink = consts.tile([128, 1], f16, tag="wsink")
            nc.vector.tensor_scalar(wsink[:], wps[:, 0:1], 0.0, None, Alu.mult)

            n0 = 0
            for ci, ch in enumerate(CHUNKS):
                nb = ch // BANK
                xks = []
                for k in range(6):
                    xk = xt_pool.tile([128, CH], f16, tag=f"x{k}", bufs=4)
                    eng = nc.sync if k % 2 == 0 else nc.gpsimd
                    eng.dma_start(xk[:, 0:ch], xt_d[k, :, n0 : n0 + ch])
                    xks.append(xk)
                x6 = xt_pool.tile([KP, CH], f16, tag="x6", bufs=4)
                nc.gpsimd.dma_start(x6[:, 0:ch], xt_d[6, 0:KP, n0 : n0 + ch])

                sp16 = sp_pool.tile([128, 2, CH], f16, tag="sp", bufs=2)
                for lc in range(2):
                    for bi in range(nb):
                        ps = psum_pool.tile([128, BANK], f32, tag="ps", bufs=8)
                        for k in range(6):
                            nc.tensor.matmul(
                                ps[:],
                                wt_sb[:, k, lc * 128 : (lc + 1) * 128],
                                xks[k][:, bi * BANK : (bi + 1) * BANK],
                                start=(k == 0),
                                stop=False,
                            )
                        nc.tensor.matmul(
                            ps[:],
                            wt_sb[0:KP, 6, lc * 128 : (lc + 1) * 128],
                            x6[:, bi * BANK : (bi + 1) * BANK],
                            start=False,
                            stop=True,
                        )
                        # fused hardtanh: (ps max -1) min 1, PSUM -> SBUF f16
                        nc.vector.tensor_scalar(
                            sp16[:, lc, bi * BANK : (bi + 1) * BANK],
                            ps[:],
                            -1.0,
                            1.0,
                            Alu.max,
                            Alu.min,
                        )
                z = tmp_pool.tile([128, 2, CH], f16, tag="z", bufs=2)
                nc.vector.tensor_tensor(
                    z[:, :, 0:ch], sp16[:, :, 0:ch], dr_sb[:, :, 0:ch], Alu.mult
                )
                th = tmp_pool.tile([128, 2, CH], f16, tag="th", bufs=2)
                nc.scalar.activation(th[:, :, 0:ch], z[:, :, 0:ch], Act.Tanh, scale=0.5)
                oq = outq_pool.tile([128, 2, 2, CH], u8, tag="oq", bufs=3)
                nc.scalar.activation(
                    oq[:, 1, :, 0:ch], th[:, :, 0:ch], Act.Square, scale=SQ255
                )
                nc.vector.tensor_scalar(
                    oq[:, 0, :, 0:ch], sp16[:, :, 0:ch], 127.5, 128.0, Alu.mult, Alu.add
                )
                nc.scalar.dma_start(
                    oq_d[:, :, :, n0 : n0 + ch].rearrange("a c p n -> p a c n"),
                    oq[:, :, :, 0:ch],
                )
                n0 += ch

    nc.compile()
    return nc


def _prep_core_x(x_flat_core):
    """[16384, 784] fp32 -> transposed fp16 [7, 128, 16384] (f on partitions).

    Row 16 of the last k-tile is the all-ones bias-fold row.
    """
    n = x_flat_core.shape[0]
    xsT16 = x_flat_core.T.astype(np.float16)  # [784, n], one strided pass
    xt = np.zeros((KT, 128, n), np.float16)
    xt[:6] = xsT16[:768].reshape(6, 128, n)
    xt[6, :16] = xsT16[768:784]
    xt[6, 16] = 1.0
    return xt


def _prep_wt(W, b):
    wt = np.zeros((KT, 128, L), np.float16)
    WT = W.T  # [784, 256]
    for k in range(6):
        wt[k] = WT[k * 128 : (k + 1) * 128]
    wt[6, :16] = WT[768:784]
    wt[6, 16] = b
    return wt


_module_cache = {}


def _get_module():
    if "m" not in _module_cache:
        _module_cache["m"] = _build_module()
    return _module_cache["m"]


def _install_ntff_hook():
    """Register the axon NTFF profiling hook missing from this image's antenv."""
    try:
        import antenv.axon_hooks  # noqa: F401

        return
    except ImportError:
        pass
    try:
        from trn_agent_boot.trn_boot import _ntff_profile_via_ctypes

        hook = _ntff_profile_via_ctypes("/opt/axon/libaxon_pjrt.so")
    except Exception:
        hook = None
    mod = types.ModuleType("antenv.axon_hooks")
    mod.get_axon_ntff_profile_hook = lambda: hook
    mod.set_axon_ntff_profile_hook = lambda h: None
    sys.modules["antenv.axon_hooks"] = mod


def _unstage(oq_raw):
    """[2, 2, 128, 16384] u8 -> (sp, gini) [16384, 256] fp32."""
    spq = np.ascontiguousarray(oq_raw[0].transpose(2, 0, 1).reshape(NROWS, L))
    giq = np.ascontiguousarray(oq_raw[1].transpose(2, 0, 1).reshape(NROWS, L))
    sp = spq.astype(np.float32)
    sp -= 127.5
    sp *= 1.0 / 127.5
    gini = giq.astype(np.float32)
    gini *= -0.5 / 255.0
    gini += 1.5
    return sp, gini


def _run(x, W, b, contribution, trace=False, tmpdir=None):
    from concourse import bass_utils

    nc = _get_module()

    x_flat = np.ascontiguousarray(x, dtype=np.float32).reshape(NCORES, NROWS, F)
    wt = _prep_wt(np.asarray(W, np.float32), np.asarray(b, np.float32))
    c = np.asarray(contribution, np.float32)
    d = np.ascontiguousarray(c[:, :, 0] - c[:, :, 1], dtype=np.float32)
    dr = np.ascontiguousarray(d.T.astype(np.float16).reshape(2, 128, T))

    with ThreadPoolExecutor(NCORES) as ex:
        xs = list(ex.map(_prep_core_x, [x_flat[i] for i in range(NCORES)]))

    if trace:
        _install_ntff_hook()
    in_maps = [{"xt": xs[i], "wt": wt, "dr": dr} for i in range(NCORES)]
    res = bass_utils.run_bass_kernel_spmd(
        nc, in_maps, core_ids=list(range(NCORES)), trace=trace, tmpdir=tmpdir
    )

    with ThreadPoolExecutor(NCORES) as ex:
        outs = list(ex.map(lambda i: _unstage(res.results[i]["oq"]), range(NCORES)))
    sp = np.concatenate([o[0] for o in outs]).reshape(B, T, L)
    gini = np.concatenate([o[1] for o in outs]).reshape(B, T, L)
    out = (sp, gini)
    return (out, res) if trace else (out, None)


def kernel(x, W, b, contribution):
    out, _ = _run(x, W, b, contribution, trace=False)
    return out


# revision 10
# speedup vs baseline: 1.1385x; 1.0159x over previous
"""Trainium2 Bass kernel for nn_Decision_Node (Linear+Hardtanh -> sp, 2-class
softmax Gini -> gini), data-parallel over 8 NeuronCores.

Math per core shard (B_s=128 of B=1024 batches, T=128, F=784, L=256, C=2):
    sp   = clip(x @ W.T + b, -1, 1)                      [N=16384, 256]
    p0   = sigmoid(sp * d),  d = contrib[...,0]-contrib[...,1]
    gini = 2 - p0^2 - p1^2 = 1.5 - 0.5*tanh(sp*d/2)^2

Device strategy (flipped layout: L on partitions, rows on free dim):
  - Stationary operand = W chunks [K=128, M=128]; moving operand =
    transposed-x tiles [K, N=512]. Bias folded as the 17th row of the
    last (K=17) contraction tile. 8 PSUM banks cycle the k=0..6
    accumulation so the PE never idles (no HAM re-throttle).
  - x is staged in DRAM chunk-major ([128, 6, ch] contiguous per chunk)
    so each chunk is ONE big DMA per queue half (24 KiB/partition lines),
    split across the sync + gpsimd queues; outputs ride the scalar queue.
  - DVE: fused hardtanh clip (PSUM drain), z = sp*d, sp uint8 quantize.
  - ACT: tanh(z/2); Square(sqrt(255)*th) -> u8 gini in one op.
  - Outputs u8, combined in one [128, (kind,lc,ch)] staging tile ->
    single DMA per chunk; host de-quantizes/transposes.
"""

import os
import sys
import types
from concurrent.futures import ThreadPoolExecutor

import numpy as np

for _p in (
    "/opt/trn_rl_repo",
    "/root/.axon_site",
    "/root/.axon_site/_ro/trn_rl_repo",
    "/root/.axon_site/_ro/pypackages",
):
    if os.path.isdir(_p) and _p not in sys.path:
        sys.path.append(_p)

B, T, F, L = 1024, 128, 784, 256
NCORES = 8
BS = B // NCORES          # batches per core
NROWS = BS * T            # 16384 rows per core
KT = 7                    # contraction tiles (784 = 6*128 + 16, + bias row)
KP = 17                   # contraction rows in the last k-tile (16 + bias)
CH = 2048                 # max rows per pipeline chunk
BANK = 512                # rows per PSUM bank / matmul free size
CHUNKS = (512, 1536) + (2048,) * 6 + (1024, 512, 512)
FILLERS = {0: 12, 1: 3, 2: 2}  # post-chunk PE filler matmuls to bridge DMA ramp


def _build_module():
    import concourse.tile as tile
    from concourse import bacc, mybir

    f32, f16, u8 = mybir.dt.float32, mybir.dt.float16, mybir.dt.uint8
    Alu = mybir.AluOpType
    Act = mybir.ActivationFunctionType

    nc = bacc.Bacc(
        "TRN2",
        target_bir_lowering=False,
        debug=False,
        enable_asserts=False,
        num_devices=NCORES,
    )
    xt_d = nc.dram_tensor("xt", [KT, 128, NROWS], f16, kind="ExternalInput").ap()
    wt_d = nc.dram_tensor("wt", [KT, 128, L], f16, kind="ExternalInput").ap()
    dr_d = nc.dram_tensor("dr", [2, 128, T], f16, kind="ExternalInput").ap()
    # combined u8 outputs: [kind (0=sp,1=gini), lc, l, n]
    oq_d = nc.dram_tensor("oq", [2, 2, 128, NROWS], u8, kind="ExternalOutput").ap()

    SQ255 = float(np.float32(np.sqrt(255.0)))

    with tile.TileContext(nc) as tc:
        with (
            tc.tile_pool(name="consts", bufs=1) as consts,
            tc.tile_pool(name="xt", bufs=3) as xt_pool,
            tc.tile_pool(name="psum", bufs=8, space="PSUM") as psum_pool,
            tc.tile_pool(name="sp", bufs=2) as sp_pool,
            tc.tile_pool(name="tmp", bufs=2) as tmp_pool,
            tc.tile_pool(name="outq", bufs=3) as outq_pool,
        ):
            wt_sb = consts.tile([128, KT, L], f16)
            nc.scalar.dma_start(wt_sb[:], wt_d.rearrange("k p l -> p k l"))
            dr_sb = consts.tile([128, 2, CH], f16)
            nc.scalar.dma_start(dr_sb[:, :, 0:T], dr_d.rearrange("c p n -> p c n"))
            # replicate d along the row axis: [*, lc, 0:128] -> [*, lc, 0:2048]
            w = T
            while w < CH:
                for lc in range(2):
                    nc.vector.tensor_scalar(
                        dr_sb[:, lc, w : 2 * w], dr_sb[:, lc, 0:w], 0.0, None, Alu.add
                    )
                w *= 2

            # PE warmup during the initial DMA wait so the HAM clock gate
            # flips to 8/8 right as real work arrives.
            wrm = consts.tile([128, BANK], f16, tag="wrm")
            nc.vector.memset(wrm[:], 0.0)
            b128 = consts.tile([128, 1], f32, tag="b128")
            nc.vector.memset(b128[:], 128.0)
            wps = psum_pool.tile([128, BANK], f32, tag="ps", bufs=8)
            for _ in range(12):
                nc.tensor.matmul(
                    wps[:], wrm[:, 0:128], wrm[:], start=True, stop=True
                )
            wsink = consts.tile([128, 1], f16, tag="wsink")
            nc.vector.tensor_scalar(wsink[:], wps[:, 0:1], 0.0, None, Alu.mult)

            n0 = 0
            for ci, ch in enumerate(CHUNKS):
                nb = ch // BANK
                xks = []
                for k in range(6):
                    xk = xt_pool.tile([128, CH], f16, tag=f"x{k}", bufs=4)
                    eng = nc.sync if k % 2 == 0 else nc.gpsimd
                    eng.dma_start(xk[:, 0:ch], xt_d[k, :, n0 : n0 + ch])
                    xks.append(xk)
                x6 = xt_pool.tile([KP, CH], f16, tag="x6", bufs=4)
                nc.gpsimd.dma_start(x6[:, 0:ch], xt_d[6, 0:KP, n0 : n0 + ch])

                sp16 = sp_pool.tile([128, 2, CH], f16, tag="sp", bufs=2)
                for lc in range(2):
                    for bi in range(nb):
                        ps = psum_pool.tile([128, BANK], f32, tag="ps", bufs=8)
                        for k in range(6):
                            nc.tensor.matmul(
                                ps[:],
                                wt_sb[:, k, lc * 128 : (lc + 1) * 128],
                                xks[k][:, bi * BANK : (bi + 1) * BANK],
                                start=(k == 0),
                                stop=False,
                            )
                        nc.tensor.matmul(
                            ps[:],
                            wt_sb[0:KP, 6, lc * 128 : (lc + 1) * 128],
                            x6[:, bi * BANK : (bi + 1) * BANK],
                            start=False,
                            stop=True,
                        )
                        # fused hardtanh: (ps max -1) min 1, PSUM -> SBUF f16
                        nc.vector.tensor_scalar(
                            sp16[:, lc, bi * BANK : (bi + 1) * BANK],
                            ps[:],
                            -1.0,
                            1.0,
                            Alu.max,
                            Alu.min,
                        )
                z = tmp_pool.tile([128, 2, CH], f16, tag="z", bufs=2)
                nc.vector.tensor_tensor(
                    z[:, :, 0:ch], sp16[:, :, 0:ch], dr_sb[:, :, 0:ch], Alu.mult
                )
                th = tmp_pool.tile([128, 2, CH], f16, tag="th", bufs=2)
                nc.scalar.activation(th[:, :, 0:ch], z[:, :, 0:ch], Act.Tanh, scale=0.5)
                oq = outq_pool.tile([128, 2, 2, CH], u8, tag="oq", bufs=3)
                nc.scalar.activation(
                    oq[:, 1, :, 0:ch], th[:, :, 0:ch], Act.Square, scale=SQ255
                )
                nc.vector.tensor_scalar(
                    oq[:, 0, :, 0:ch], sp16[:, :, 0:ch], 127.5, 128.0, Alu.mult, Alu.add
                )
                nc.scalar.dma_start(
                    oq_d[:, :, :, n0 : n0 + ch].rearrange("a c p n -> p a c n"),
                    oq[:, :, :, 0:ch],
                )
                for _ in range(FILLERS.get(ci, 0)):
                    fps = psum_pool.tile([128, BANK], f32, tag="ps", bufs=8)
                    nc.tensor.matmul(fps[:], wrm[:, 0:128], wrm[:], start=True, stop=True)
                n0 += ch

    nc.compile()
    return nc


def _prep_core_x(x_flat_core):
    """[16384, 784] fp32 -> transposed fp16 [7, 128, 16384] (f on partitions).

    Row 16 of the last k-tile is the all-ones bias-fold row.
    """
    n = x_flat_core.shape[0]
    xsT16 = x_flat_core.T.astype(np.float16)  # [784, n], one strided pass
    xt = np.zeros((KT, 128, n), np.float16)
    xt[:6] = xsT16[:768].reshape(6, 128, n)
    xt[6, :16] = xsT16[768:784]
    xt[6, 16] = 1.0
    return xt


def _prep_wt(W, b):
    wt = np.zeros((KT, 128, L), np.float16)
    WT = W.T  # [784, 256]
    for k in range(6):
        wt[k] = WT[k * 128 : (k + 1) * 128]
    wt[6, :16] = WT[768:784]
    wt[6, 16] = b
    return wt


_module_cache = {}


def _get_module():
    if "m" not in _module_cache:
        _module_cache["m"] = _build_module()
    return _module_cache["m"]


def _install_ntff_hook():
    """Register the axon NTFF profiling hook missing from this image's antenv."""
    try:
        import antenv.axon_hooks  # noqa: F401

        return
    except ImportError:
        pass
    try:
        from trn_agent_boot.trn_boot import _ntff_profile_via_ctypes

        hook = _ntff_profile_via_ctypes("/opt/axon/libaxon_pjrt.so")
    except Exception:
        hook = None
    mod = types.ModuleType("antenv.axon_hooks")
    mod.get_axon_ntff_profile_hook = lambda: hook
    mod.set_axon_ntff_profile_hook = lambda h: None
    sys.modules["antenv.axon_hooks"] = mod


def _unstage(oq_raw):
    """[2, 2, 128, 16384] u8 -> (sp, gini) [16384, 256] fp32."""
    spq = np.ascontiguousarray(oq_raw[0].transpose(2, 0, 1).reshape(NROWS, L))
    giq = np.ascontiguousarray(oq_raw[1].transpose(2, 0, 1).reshape(NROWS, L))
    sp = spq.astype(np.float32)
    sp -= 127.5
    sp *= 1.0 / 127.5
    gini = giq.astype(np.float32)
    gini *= -0.5 / 255.0
    gini += 1.5
    return sp, gini


def _run(x, W, b, contribution, trace=False, tmpdir=None):
    from concourse import bass_utils

    nc = _get_module()

    x_flat = np.ascontiguousarray(x, dtype=np.float32).reshape(NCORES, NROWS, F)
    wt = _prep_wt(np.asarray(W, np.float32), np.asarray(b, np.float32))
    c = np.asarray(contribution, np.float32)
    d = np.ascontiguousarray(c[:, :, 0] - c[:, :, 1], dtype=np.float32)
    dr = np.ascontiguousarray(d.T.astype(np.float16).reshape(2, 128, T))

    with ThreadPoolExecutor(NCORES) as ex:
        xs = list(ex.map(_prep_core_x, [x_flat[i] for i in range(NCORES)]))

    if trace:
        _install_ntff_hook()
    in_maps = [{"xt": xs[i], "wt": wt, "dr": dr} for i in range(NCORES)]
    res = bass_utils.run_bass_kernel_spmd(
        nc, in_maps, core_ids=list(range(NCORES)), trace=trace, tmpdir=tmpdir
    )

    with ThreadPoolExecutor(NCORES) as ex:
        outs = list(ex.map(lambda i: _unstage(res.results[i]["oq"]), range(NCORES)))
    sp = np.concatenate([o[0] for o in outs]).reshape(B, T, L)
    gini = np.concatenate([o[1] for o in outs]).reshape(B, T, L)
    out = (sp, gini)
    return (out, res) if trace else (out, None)


def kernel(x, W, b, contribution):
    out, _ = _run(x, W, b, contribution, trace=False)
    return out
